# revision 2
# baseline (speedup 1.0000x reference)
"""Trainium2 Bass kernel for KL-divergence 1-NN label lookup (AnchorStore).

reference:
    self[k]  = mean_d a[k,d]*log a[k,d]
    cross    = einsum('kd,bd->kb', a, log q) / D
    kl[b,k]  = self[k] - cross[k,b]
    out[b]   = queue_label[argmin_k kl[b,k]]

Strategy (8 NeuronCores, D-sharded, fp16 operands), v2:
    Each core owns a D-slice (padded with 1.0 so log()=0 contributes
    nothing), shipped as fp16 in d-tile-major layout per k-pass.
    Working in SUM units (scale-invariant for argmin):
        m[b,k] = sum_d lq[d,b]*at[d,k] - sum_d at[d,k]*log(at[d,k])
    K is split into P=5 passes [512,512,512,256,256]; each pass gets its
    own ReduceScatter(add) launched right after its drain, so the CC
    engine pipelines under compute and only a small 256-col RS remains
    on the tail.
      - TensorE: stationary lq tiles [128d,128b] x moving at [128d,W]
        accumulate cross into PSUM; the -self term accumulates via a
        (-1)-stationary x per-batch unit tiles: DVE computes fp16 pair
        sums (tt[2i]+tt[2i+1], identical pairing to v1), and the Pool
        engine (otherwise idle) tree-adds the pairs of each batch into
        one unit tile, cutting the srep matmul column count ~3x.
      - ScalarE computes log() (Ln activation) in large batches.
      - Drain per pass: m = pk + srep -> DRAM -> ReduceScatter.
    Pipeline ramp: pass 0 starts with small d-batches and pass 1's
    first two batches are interleaved into the ramp.
    Tail: msum loads for passes 0..P-2 are pinned behind the last drain
    (WAW copy) and their argmax partials (value + label via the
    is_equal trick) run under the last RS; only the last slice's
    epilogue and a P-column combine remain after it.
"""

import os
import sys

import numpy as np

sys.path.insert(0, "/opt/trn_rl_repo")

from concourse import bacc, bass, mybir, tile  # noqa: E402
from concourse import bass_utils  # noqa: E402

K = 2048
B = 256
D = 50257
NCORES = 8
NT = 50             # d-tiles of 128 per core (padded)
DSH = NT * 128      # 6400
BS = B // NCORES    # 32 queries per core after ReduceScatter
F32 = mybir.dt.float32
F16 = mybir.dt.float16

PW = [512, 512, 512, 256, 256]          # per-pass k widths
POFF = [0, 512, 1024, 1536, 1792]       # pass k offsets
P = len(PW)

# d-tile batches: even boundaries keep the fp16 pair sums (2i,2i+1)
# identical to v1; the two trailing 1-pair batches keep the per-pass
# drain chain short (their srep unit needs no Pool adds).
BATCHES = [(0, 2), (2, 6), (6, 14), (14, 22), (22, 30), (30, 38),
           (38, 46), (46, 48), (48, 50)]


def build(mm_dtype=F16, warm_cc=True):
    """Build the SPMD Bass graph for one core (all cores identical)."""
    nc = bacc.Bacc(
        "TRN2", target_bir_lowering=False, debug=False, num_devices=NCORES
    )
    at_d = [
        nc.dram_tensor(f"at{p}", [128, NT, PW[p]], mm_dtype,
                       kind="ExternalInput")
        for p in range(P)
    ]
    qt_d = nc.dram_tensor("qt", [128, NT, B], mm_dtype, kind="ExternalInput")
    lab_d = nc.dram_tensor("lab1", [BS, K], F32, kind="ExternalInput")
    out_d = nc.dram_tensor("out", [BS], mybir.dt.int32, kind="ExternalOutput")

    LN = mybir.ActivationFunctionType.Ln
    AX = mybir.AxisListType.X
    OP = mybir.AluOpType

    # q chunks for lq computation (front chunks small for fast start)
    qch = [(0, 2), (2, 6), (6, 14), (14, 26), (26, 38), (38, NT)]

    with tile.TileContext(nc) as tc:
        with (
            tc.tile_pool(name="const", bufs=1) as constp,
            tc.tile_pool(name="lqp", bufs=1) as lqp,
            tc.tile_pool(name="qinp", bufs=2) as qinp,
            tc.tile_pool(name="atp", bufs=4) as atp,
            tc.tile_pool(name="latp", bufs=3) as latp,
            tc.tile_pool(name="ttp", bufs=3) as ttp,
            tc.tile_pool(name="tpp", bufs=6) as tpp,
            tc.tile_pool(name="qup", bufs=8) as qup,
            tc.tile_pool(name="msbp", bufs=2) as msbp,
            tc.tile_pool(name="epp", bufs=1) as epp,
            tc.tile_pool(name="psp", bufs=1, space="PSUM") as psp,
            tc.tile_pool(name="dramp", bufs=1, space="DRAM") as dramp,
        ):
            # --- constants / warmup ---------------------------------
            wdma_d = dramp.tile([1, 16], F32, name="wdma_d", bufs=1)
            for eng in (nc.sync, nc.scalar, nc.gpsimd):
                wdma_s = constp.tile([1, 16], F32, name=f"wdma_{eng.engine}")
                eng.dma_start(wdma_s[:], wdma_d[:])

            # --- lq = log(query^T), fp16, resident -------------------
            lq = lqp.tile([128, NT, B], mm_dtype)
            qsb = []
            for ci, (c0_, c1_) in enumerate(qch):
                qtile = qinp.tile(
                    [128, c1_ - c0_, B], mm_dtype, name=f"qtile_{ci}",
                    tag="qtile",
                )
                qsb.append((qtile, c0_, c1_))
            dum = constp.tile([128, 16], F32)
            nc.gpsimd.memset(dum[:], 1.0)
            dumo = constp.tile([128, 16], F32)
            nc.scalar.activation(dumo[:], dum[:], LN)
            negones_f = constp.tile([128, 128], F32)
            nc.gpsimd.memset(negones_f[:], -1.0)
            negones = constp.tile([128, 128], mm_dtype)
            nc.vector.tensor_copy(negones[:], negones_f[:])

            # qt triggers go first on gpsimd: the first matmul waits
            # on qt chunk 0 -> lq.
            for qtile, c0_, c1_ in qsb[:3]:
                nc.gpsimd.dma_start(qtile[:], qt_d[:, c0_:c1_, :])
            nc.scalar.activation(
                lq[:, qch[0][0]:qch[0][1], :], qsb[0][0][:], LN
            )

            if warm_cc:
                # Tiny dummy collective early: pre-warms ncfw/credit
                # state and is the cross-core rendezvous, well before
                # the first real ReduceScatter (~45us in).
                w_in = dramp.tile([1, 64], F32)
                w_out = dramp.tile([NCORES, 64], F32)
                w_sb = constp.tile([1, 64], F32)
                nc.gpsimd.memset(w_sb[:], 1.0)
                nc.gpsimd.dma_start(w_in[:], w_sb[:])
                nc.gpsimd.collective_compute(
                    "AllGather",
                    OP.bypass,
                    replica_groups=[list(range(NCORES))],
                    ins=[w_in.opt()],
                    outs=[w_out.opt()],
                )

            # ~40 dummy matmuls while the PE waits for the first lq
            # tile: the HAM clock gate needs ~3.4us of sustained PE
            # activity to lift the 1.2 GHz cold throttle.
            warm_ps = psp.tile([128, 128], F32, name="warm_ps")
            for wi in range(40):
                nc.tensor.matmul(
                    warm_ps[:], negones[:], negones[:],
                    start=(wi == 0), stop=(wi == 39),
                )

            lab1 = constp.tile([BS, K], F32)
            nc.gpsimd.dma_start(lab1[:], lab_d[:])

            # --- PSUM accumulators (parity double-buffered) ----------
            pk = {}
            srep = {}
            for par in range(2):
                for bti in range(2):
                    pk[(par, bti)] = psp.tile(
                        [128, 512], F32, name=f"pk_{par}_{bti}",
                        tag=f"pk_{par}_{bti}",
                    )
                srep[par] = psp.tile(
                    [128, 512], F32, name=f"srep_{par}", tag=f"srep_{par}",
                )

            vcat = epp.tile([BS, P], F32, bufs=1)
            lcat = epp.tile([BS, P], F32, bufs=1)

            qt_dma_emitted = 1  # chunk 0 already emitted

            ar = [dramp.tile([B, PW[p]], F32, name=f"ar_{p}", bufs=1)
                  for p in range(P)]
            rs = [dramp.tile([BS, PW[p]], F32, name=f"rs_{p}", bufs=1)
                  for p in range(P)]

            # Explicit (ps, bi) schedule: pass-1's first two batches
            # are interleaved into pass-0's ramp.
            sched = []
            for ps_i in range(P):
                for bi_i, tb in enumerate(BATCHES):
                    sched.append(
                        (ps_i, bi_i, tb, bi_i == len(BATCHES) - 1)
                    )
            n0 = len(BATCHES)
            p1b0 = sched.pop(n0)
            p1b1 = sched.pop(n0)
            sched.insert(2, p1b0)
            sched.insert(5, p1b1)

            pend_srep_ps = {ps_i: [] for ps_i in range(P)}
            ramp_i = 0
            for ps, bi, (tb0, tb1), last_b in sched:
                par = ps % 2
                W = PW[ps]
                pend_srep = pend_srep_ps[ps]

                def flush_srep(fin):
                    u_, first_ = pend_srep.pop(0)
                    nc.tensor.matmul(
                        srep[par][:, :W],
                        negones[:],
                        u_,
                        start=first_,
                        stop=fin,
                    )

                n = tb1 - tb0
                npair = n // 2
                att = atp.tile(
                    [128, 8, 512], mm_dtype, name=f"att_{ps}_{bi}",
                    tag="att",
                )
                nc.sync.dma_start(
                    att[:, :n, :W], at_d[ps][:, tb0:tb1, :]
                )
                # later qt chunks follow the early att batches on sync
                if ramp_i % 2 == 0 and 3 <= ramp_i // 2 + 2 < len(qsb):
                    qtile, c0_, c1_ = qsb[ramp_i // 2 + 2]
                    nc.sync.dma_start(qtile[:], qt_d[:, c0_:c1_, :])
                latt = latp.tile(
                    [128, 8, 512], mm_dtype,
                    name=f"latt_{ps}_{bi}", tag="latt",
                )
                nc.scalar.activation(latt[:, :n, :W], att[:, :n, :W], LN)
                # trickle in remaining lq activations between the
                # early batches
                if (
                    qt_dma_emitted < len(qch)
                    and (qt_dma_emitted <= 2
                         or qt_dma_emitted <= ramp_i // 2 + 2)
                ):
                    qtile, c0_, c1_ = qsb[qt_dma_emitted]
                    nc.scalar.activation(
                        lq[:, c0_:c1_, :], qtile[:], LN
                    )
                    qt_dma_emitted += 1
                ramp_i += 1
                tt = ttp.tile(
                    [128, 8, 512], mm_dtype, name=f"tt_{ps}_{bi}",
                    tag="tt",
                )
                nc.vector.tensor_tensor(
                    tt[:, :n, :W], att[:, :n, :W], latt[:, :n, :W],
                    op=OP.mult,
                )
                # before the last batch's cross matmuls, flush all
                # pending srep matmuls
                if last_b:
                    while pend_srep:
                        flush_srep(False)
                # cross matmuls
                for j in range(n):
                    t = tb0 + j
                    for bti in range(2):
                        lhs = lq[:, t, bti * 128:(bti + 1) * 128]
                        nc.tensor.matmul(
                            pk[(par, bti)][:, :W],
                            lhs,
                            att[:, j, :W],
                            start=(t == 0),
                            stop=(t == NT - 1),
                        )
                # self term: fp16 pair sums on DVE (identical pairing
                # to v1), then Pool tree-adds the batch's pairs into a
                # single unit tile for the srep matmul.
                tp = tpp.tile(
                    [128, 4, 512], mm_dtype, name=f"tp_{ps}_{bi}",
                    tag="tp",
                )
                nc.vector.tensor_tensor(
                    tp[:, :npair, :W],
                    tt[:, 0:n:2, :W],
                    tt[:, 1:n:2, :W],
                    op=OP.add,
                )
                if npair == 1:
                    unit = tp[:, 0, :W]
                elif npair == 2:
                    uq = qup.tile(
                        [128, 512], mm_dtype, name=f"uq_{ps}_{bi}",
                        tag="uq",
                    )
                    nc.gpsimd.tensor_tensor(
                        uq[:, :W], tp[:, 0, :W], tp[:, 1, :W], op=OP.add
                    )
                    unit = uq[:, :W]
                else:  # npair == 4
                    ua = qup.tile(
                        [128, 512], mm_dtype, name=f"ua_{ps}_{bi}",
                        tag="ua",
                    )
                    ub = qup.tile(
                        [128, 512], mm_dtype, name=f"ub_{ps}_{bi}",
                        tag="ub",
                    )
                    uq = qup.tile(
                        [128, 512], mm_dtype, name=f"uq_{ps}_{bi}",
                        tag="uq",
                    )
                    nc.gpsimd.tensor_tensor(
                        ua[:, :W], tp[:, 0, :W], tp[:, 1, :W], op=OP.add
                    )
                    nc.gpsimd.tensor_tensor(
                        ub[:, :W], tp[:, 2, :W], tp[:, 3, :W], op=OP.add
                    )
                    nc.gpsimd.tensor_tensor(
                        uq[:, :W], ua[:, :W], ub[:, :W], op=OP.add
                    )
                    unit = uq[:, :W]
                pend_srep.append((unit, bi == 0))

                if len(pend_srep) > 2:
                    flush_srep(False)
                if last_b:
                    while len(pend_srep) > 1:
                        flush_srep(False)
                    flush_srep(True)

                if ps == 0 and bi == 0:
                    for wi in range(24):
                        nc.tensor.matmul(
                            warm_ps[:], negones[:], negones[:],
                            start=(wi == 0), stop=(wi == 23),
                        )
                if not last_b:
                    continue
                # --- drain pass ps: PSUM -> SBUF -> DRAM -> RS -------
                srep_sb = msbp.tile(
                    [128, 512], F32, name=f"srep_sb_{ps}", tag="srep_sb",
                )
                nc.vector.tensor_copy(srep_sb[:, :W], srep[par][:, :W])
                for bti in range(2):
                    m_sb = msbp.tile(
                        [128, 512], F32, name=f"m_sb_{ps}_{bti}",
                        tag=f"m_sb{bti}",
                    )
                    nc.vector.tensor_tensor(
                        m_sb[:, :W], pk[(par, bti)][:, :W],
                        srep_sb[:, :W], op=OP.add,
                    )
                    nc.gpsimd.dma_start(
                        ar[ps][bti * 128:(bti + 1) * 128, :],
                        m_sb[:, :W],
                    )
                    m_sb_last = m_sb
                nc.gpsimd.collective_compute(
                    "ReduceScatter",
                    OP.add,
                    replica_groups=[list(range(NCORES))],
                    ins=[ar[ps].opt()],
                    outs=[rs[ps].opt()],
                )

            # --- tail: msum loads + per-slice epilogues -------------
            # msum loads for passes 0..P-2 are pinned behind the last
            # pass's drain via a tiny WAW copy so the scheduler can
            # never hoist them (their RS's are long done by then); they
            # ride the sync queue, idle once att loads are done.
            msum = []
            for p_i in range(P):
                mt = epp.tile([BS, PW[p_i]], F32, name=f"msum_{p_i}",
                              bufs=1)
                msum.append(mt)
            for p_i in range(P):
                nc.vector.tensor_copy(
                    msum[p_i][0:BS, 0:1], m_sb_last[0:BS, 0:1]
                )
                nc.sync.dma_start(msum[p_i][:], rs[p_i][:])

            def emit_epi(col, mt, w, lab_off):
                nc.vector.tensor_reduce(
                    vcat[:, col:col + 1], mt[:], axis=AX, op=OP.max
                )
                cand = epp.tile(
                    [BS, w], F32, name=f"cand_{col}", tag=f"cand_{col}",
                )
                nc.vector.scalar_tensor_tensor(
                    cand[:], mt[:], vcat[:, col:col + 1],
                    lab1[:, lab_off:lab_off + w],
                    op0=OP.is_equal, op1=OP.mult,
                )
                nc.vector.tensor_reduce(
                    lcat[:, col:col + 1], cand[:], axis=AX, op=OP.max
                )

            for p_i in range(P):
                emit_epi(p_i, msum[p_i], PW[p_i], POFF[p_i])

            # --- final combine across passes ------------------------
            vg = epp.tile([BS, 1], F32, bufs=1)
            nc.vector.tensor_reduce(vg[:], vcat[:], axis=AX, op=OP.max)
            candp = epp.tile([BS, P], F32, bufs=1)
            nc.vector.scalar_tensor_tensor(
                candp[:], vcat[:], vg[:], lcat[:],
                op0=OP.is_equal, op1=OP.mult,
            )
            lmax = epp.tile([BS, 1], F32, bufs=1)
            nc.vector.tensor_reduce(lmax[:], candp[:], axis=AX, op=OP.max)
            labf = epp.tile([BS, 1], F32, bufs=1)
            nc.vector.tensor_scalar_add(labf[:], lmax[:], -1.0)
            labi = epp.tile([BS, 1], mybir.dt.int32, bufs=1)
            nc.vector.tensor_copy(labi[:], labf[:])
            nc.scalar.dma_start(out_d[:], labi[:])

    nc.compile()
    return nc


def shard_inputs(query, queue_anchor, queue_label, dsh=DSH, d_real=D):
    """Host-side layout prep: pad D with 1.0 (log 1 = 0); at in
    pass-major fp16 layout (one tensor per k-pass, [128, NT, W]),
    qt tile-major [128, NT, B]; label row replicated."""
    np_dt = np.float16
    q = np.asarray(query, np.float32)
    a = np.asarray(queue_anchor, np.float32)
    lab1 = (np.asarray(queue_label).astype(np.float32) + 1.0)[None, :]
    lab1 = np.ascontiguousarray(np.broadcast_to(lab1, (BS, lab1.shape[1])))
    in_maps = []
    for c in range(NCORES):
        lo = c * dsh
        hi = min((c + 1) * dsh, d_real)
        at = np.ones((dsh, a.shape[0]), np_dt)
        qt = np.ones((dsh, q.shape[0]), np_dt)
        if hi > lo:
            at[: hi - lo, :] = a[:, lo:hi].T.astype(np_dt)
            qt[: hi - lo, :] = q[:, lo:hi].T.astype(np_dt)
        # at: [dsh, K] -> per pass [128, NT, W] (tile-major, contiguous)
        at4 = at.reshape(NT, 128, K).transpose(1, 0, 2)  # [128, NT, K]
        m = {}
        for p in range(P):
            m[f"at{p}"] = np.ascontiguousarray(
                at4[:, :, POFF[p]:POFF[p] + PW[p]]
            )
        # qt: [dsh, B] -> tile-major [128, NT, B]
        m["qt"] = np.ascontiguousarray(
            qt.reshape(NT, 128, -1).transpose(1, 0, 2)
        )
        m["lab1"] = lab1
        in_maps.append(m)
    return in_maps


def unshard_out(per_core_outs, split_rs=False):
    """Reassemble the 8 cores' 32-label slices into the [256] output."""
    return np.concatenate([np.asarray(o) for o in per_core_outs])


_NC_CACHE = {}


def _split_rs_active():
    return False


def _get_nc():
    key = ("v2",)
    if key not in _NC_CACHE:
        _NC_CACHE[key] = build()
    return _NC_CACHE[key]


def kernel(query, queue_anchor, queue_label):
    nc = _get_nc()
    in_maps = shard_inputs(query, queue_anchor, queue_label)
    res = bass_utils.run_bass_kernel_spmd(
        nc, in_maps, core_ids=list(range(NCORES))
    )
    out = unshard_out([res.results[i]["out"] for i in range(NCORES)])
    return out.astype(np.asarray(queue_label).dtype)


# revision 4
# speedup vs baseline: 1.2451x; 1.2451x over previous
"""Trainium2 Bass kernel for KL-divergence 1-NN label lookup (AnchorStore).

reference:
    self[k]  = mean_d a[k,d]*log a[k,d]
    cross    = einsum('kd,bd->kb', a, log q) / D
    kl[b,k]  = self[k] - cross[k,b]
    out[b]   = queue_label[argmin_k kl[b,k]]

Strategy (8 NeuronCores, D-sharded, fp16 operands), v3:
    Each core owns a D-slice (padded with 1.0 so log()=0 contributes
    nothing), shipped as fp16 in tile-major layout per k-pass.
    Working in SUM units (scale-invariant for argmin):
        m[b,k] = sum_d lq[d,b]*at[d,k] - sum_d at[d,k]*log(at[d,k])
    K is split into P=5 passes [512,512,512,384,128]. Collectives:
    passes {0,1} share one merged ReduceScatter(add) (launched ~75us,
    fully overlapped), then per-pass RS for 2/3/4 — the tail RS is only
    128 cols. All collective payloads stay fp32 (the argmin gaps are
    ~1e-5 relative; fp16 payloads flip labels).
      - TensorE: stationary lq tiles [128d,128b] x moving at [128d,W]
        accumulate cross into PSUM; the -self term accumulates via a
        (-1)-stationary matmul over per-batch unit tiles: DVE computes
        fp16 pair sums (tt[2i]+tt[2i+1]) and tree-adds each batch's
        pairs into one unit, cutting srep matmul columns ~3x vs
        per-pair. All unit arithmetic is bit-identical to the verified
        v2 run (same fp16 adds, same order) — only the engine moved
        from Pool (which saturated) back to DVE.
      - ScalarE computes log() (Ln activation) in large batches.
      - Drain per pass: one scalar_tensor_tensor per b-half folds
        (pk + 0) + srep -> SBUF -> DRAM -> ReduceScatter.
    Pipeline ramp: pass 0 starts with small d-batches and pass 1's
    first two batches are interleaved into the ramp.
    Tail: msum loads pinned behind the last drain (WAW copy) so the
    scheduler can't hoist them; argmax partials for slices 0..3 run
    under the tail RS via the is_equal trick; a P-column combine emits
    the 32 int32 labels per core; host concats.
"""

import os
import sys

import numpy as np

sys.path.insert(0, "/opt/trn_rl_repo")

from concourse import bacc, bass, mybir, tile  # noqa: E402
from concourse import bass_utils  # noqa: E402

K = 2048
B = 256
D = 50257
NCORES = 8
NT = 50             # d-tiles of 128 per core (padded)
DSH = NT * 128      # 6400
BS = B // NCORES    # 32 queries per core after ReduceScatter
F32 = mybir.dt.float32
F16 = mybir.dt.float16

PW = [512, 512, 512, 384, 128]          # per-pass k widths
POFF = [0, 512, 1024, 1536, 1920]       # pass k offsets
P = len(PW)

# d-tile batches: even boundaries keep the fp16 pair sums (2i,2i+1)
# identical to v1/v2; trailing 1-pair batches keep drain chains short.
BATCHES = [(0, 2), (2, 6), (6, 14), (14, 22), (22, 30), (30, 38),
           (38, 46), (46, 48), (48, 50)]


def build(mm_dtype=F16, warm_cc=True):
    """Build the SPMD Bass graph for one core (all cores identical)."""
    nc = bacc.Bacc(
        "TRN2", target_bir_lowering=False, debug=False, num_devices=NCORES
    )
    at_d = [
        nc.dram_tensor(f"at{p}", [128, NT, PW[p]], mm_dtype,
                       kind="ExternalInput")
        for p in range(P)
    ]
    qt_d = nc.dram_tensor("qt", [128, NT, B], mm_dtype, kind="ExternalInput")
    lab_d = nc.dram_tensor("lab1", [BS, K], F32, kind="ExternalInput")
    out_d = nc.dram_tensor("out", [BS], mybir.dt.int32, kind="ExternalOutput")

    LN = mybir.ActivationFunctionType.Ln
    AX = mybir.AxisListType.X
    OP = mybir.AluOpType

    # q chunks for lq computation (front chunks small for fast start)
    qch = [(0, 2), (2, 6), (6, 14), (14, 26), (26, 38), (38, NT)]

    with tile.TileContext(nc) as tc:
        with (
            tc.tile_pool(name="const", bufs=1) as constp,
            tc.tile_pool(name="lqp", bufs=1) as lqp,
            tc.tile_pool(name="qinp", bufs=2) as qinp,
            tc.tile_pool(name="atp", bufs=4) as atp,
            tc.tile_pool(name="latp", bufs=3) as latp,
            tc.tile_pool(name="ttp", bufs=3) as ttp,
            tc.tile_pool(name="tpp", bufs=6) as tpp,
            tc.tile_pool(name="qup", bufs=6) as qup,
            tc.tile_pool(name="msbp", bufs=2) as msbp,
            tc.tile_pool(name="epp", bufs=1) as epp,
            tc.tile_pool(name="psp", bufs=1, space="PSUM") as psp,
            tc.tile_pool(name="dramp", bufs=1, space="DRAM") as dramp,
        ):
            # --- constants / warmup ---------------------------------
            wdma_d = dramp.tile([1, 16], F32, name="wdma_d", bufs=1)
            for eng in (nc.sync, nc.scalar, nc.gpsimd):
                wdma_s = constp.tile([1, 16], F32, name=f"wdma_{eng.engine}")
                eng.dma_start(wdma_s[:], wdma_d[:])

            # --- lq = log(query^T), fp16, resident -------------------
            lq = lqp.tile([128, NT, B], mm_dtype)
            qsb = []
            for ci, (c0_, c1_) in enumerate(qch):
                qtile = qinp.tile(
                    [128, c1_ - c0_, B], mm_dtype, name=f"qtile_{ci}",
                    tag="qtile",
                )
                qsb.append((qtile, c0_, c1_))
            dum = constp.tile([128, 16], F32)
            nc.gpsimd.memset(dum[:], 1.0)
            dumo = constp.tile([128, 16], F32)
            nc.scalar.activation(dumo[:], dum[:], LN)
            negones_f = constp.tile([128, 128], F32)
            nc.gpsimd.memset(negones_f[:], -1.0)
            negones = constp.tile([128, 128], mm_dtype)
            nc.vector.tensor_copy(negones[:], negones_f[:])

            # qt triggers go first on gpsimd: the first matmul waits
            # on qt chunk 0 -> lq.
            for qtile, c0_, c1_ in qsb[:3]:
                nc.gpsimd.dma_start(qtile[:], qt_d[:, c0_:c1_, :])
            nc.scalar.activation(
                lq[:, qch[0][0]:qch[0][1], :], qsb[0][0][:], LN
            )

            if warm_cc:
                # Tiny dummy collective: rendezvous + ncfw/credit warm
                # well before the first real ReduceScatter.
                w_in = dramp.tile([1, 64], F32)
                w_out = dramp.tile([NCORES, 64], F32)
                w_sb = constp.tile([1, 64], F32)
                nc.gpsimd.memset(w_sb[:], 1.0)
                nc.gpsimd.dma_start(w_in[:], w_sb[:])
                nc.gpsimd.collective_compute(
                    "AllGather",
                    OP.bypass,
                    replica_groups=[list(range(NCORES))],
                    ins=[w_in.opt()],
                    outs=[w_out.opt()],
                )

            # ~40 dummy matmuls while the PE waits for the first lq
            # tile: the HAM clock gate needs ~3.4us of sustained PE
            # activity to lift the 1.2 GHz cold throttle.
            warm_ps = psp.tile([128, 128], F32, name="warm_ps")
            for wi in range(40):
                nc.tensor.matmul(
                    warm_ps[:], negones[:], negones[:],
                    start=(wi == 0), stop=(wi == 39),
                )

            lab1 = constp.tile([BS, K], F32)
            nc.gpsimd.dma_start(lab1[:], lab_d[:])

            # --- PSUM accumulators (parity double-buffered) ----------
            pk = {}
            srep = {}
            for par in range(2):
                for bti in range(2):
                    pk[(par, bti)] = psp.tile(
                        [128, 512], F32, name=f"pk_{par}_{bti}",
                        tag=f"pk_{par}_{bti}",
                    )
                srep[par] = psp.tile(
                    [128, 512], F32, name=f"srep_{par}", tag=f"srep_{par}",
                )

            vcat = epp.tile([BS, P], F32, bufs=1)
            lcat = epp.tile([BS, P], F32, bufs=1)

            qt_dma_emitted = 1  # chunk 0 already emitted

            # Collective groups: {0,1} merged (1MB), then per-pass.
            ar_a = dramp.tile([B, PW[0] + PW[1]], F32, name="ar_a", bufs=1)
            rs_a = dramp.tile([BS, PW[0] + PW[1]], F32, name="rs_a", bufs=1)
            ar = {0: ar_a, 1: ar_a}
            aroff = {0: 0, 1: PW[0]}
            rs_t = {1: rs_a}
            for p_i in range(2, P):
                ar[p_i] = dramp.tile([B, PW[p_i]], F32, name=f"ar_{p_i}",
                                     bufs=1)
                aroff[p_i] = 0
                rs_t[p_i] = dramp.tile([BS, PW[p_i]], F32, name=f"rs_{p_i}",
                                       bufs=1)

            # Explicit (ps, bi) schedule: pass-1's first two batches
            # are interleaved into pass-0's ramp.
            sched = []
            for ps_i in range(P):
                for bi_i, tb in enumerate(BATCHES):
                    sched.append(
                        (ps_i, bi_i, tb, bi_i == len(BATCHES) - 1)
                    )
            n0 = len(BATCHES)
            p1b0 = sched.pop(n0)
            p1b1 = sched.pop(n0)
            sched.insert(2, p1b0)
            sched.insert(5, p1b1)

            pend_srep_ps = {ps_i: [] for ps_i in range(P)}
            ramp_i = 0
            for ps, bi, (tb0, tb1), last_b in sched:
                par = ps % 2
                W = PW[ps]
                pend_srep = pend_srep_ps[ps]

                def flush_srep(fin):
                    u_, first_ = pend_srep.pop(0)
                    nc.tensor.matmul(
                        srep[par][:, :W],
                        negones[:],
                        u_,
                        start=first_,
                        stop=fin,
                    )

                n = tb1 - tb0
                npair = n // 2
                att = atp.tile(
                    [128, 8, 512], mm_dtype, name=f"att_{ps}_{bi}",
                    tag="att",
                )
                nc.sync.dma_start(
                    att[:, :n, :W], at_d[ps][:, tb0:tb1, :]
                )
                # later qt chunks follow the early att batches on sync
                if ramp_i % 2 == 0 and 3 <= ramp_i // 2 + 2 < len(qsb):
                    qtile, c0_, c1_ = qsb[ramp_i // 2 + 2]
                    nc.sync.dma_start(qtile[:], qt_d[:, c0_:c1_, :])
                latt = latp.tile(
                    [128, 8, 512], mm_dtype,
                    name=f"latt_{ps}_{bi}", tag="latt",
                )
                nc.scalar.activation(latt[:, :n, :W], att[:, :n, :W], LN)
                # trickle in remaining lq activations between the
                # early batches
                if (
                    qt_dma_emitted < len(qch)
                    and (qt_dma_emitted <= 2
                         or qt_dma_emitted <= ramp_i // 2 + 2)
                ):
                    qtile, c0_, c1_ = qsb[qt_dma_emitted]
                    nc.scalar.activation(
                        lq[:, c0_:c1_, :], qtile[:], LN
                    )
                    qt_dma_emitted += 1
                ramp_i += 1
                tt = ttp.tile(
                    [128, 8, 512], mm_dtype, name=f"tt_{ps}_{bi}",
                    tag="tt",
                )
                nc.vector.tensor_tensor(
                    tt[:, :n, :W], att[:, :n, :W], latt[:, :n, :W],
                    op=OP.mult,
                )
                # before the last batch's cross matmuls, flush all
                # pending srep matmuls
                if last_b:
                    while pend_srep:
                        flush_srep(False)
                # cross matmuls
                for j in range(n):
                    t = tb0 + j
                    for bti in range(2):
                        lhs = lq[:, t, bti * 128:(bti + 1) * 128]
                        nc.tensor.matmul(
                            pk[(par, bti)][:, :W],
                            lhs,
                            att[:, j, :W],
                            start=(t == 0),
                            stop=(t == NT - 1),
                        )
                # self term: fp16 pair sums + per-batch unit tree, all
                # on DVE, bit-identical to the verified v2 numerics.
                tp = tpp.tile(
                    [128, 4, 512], mm_dtype, name=f"tp_{ps}_{bi}",
                    tag="tp",
                )
                for i in range(npair):
                    nc.vector.tensor_tensor(
                        tp[:, i, :W], tt[:, 2 * i, :W],
                        tt[:, 2 * i + 1, :W], op=OP.add,
                    )
                if npair == 1:
                    unit = tp[:, 0, :W]
                elif npair == 2:
                    uq = qup.tile(
                        [128, 512], mm_dtype, name=f"uq_{ps}_{bi}",
                        tag="uq",
                    )
                    nc.vector.tensor_tensor(
                        uq[:, :W], tp[:, 0, :W], tp[:, 1, :W], op=OP.add
                    )
                    unit = uq[:, :W]
                else:  # npair == 4
                    ua = qup.tile(
                        [128, 512], mm_dtype, name=f"ua_{ps}_{bi}",
                        tag="ua",
                    )
                    ub = qup.tile(
                        [128, 512], mm_dtype, name=f"ub_{ps}_{bi}",
                        tag="ub",
                    )
                    uq = qup.tile(
                        [128, 512], mm_dtype, name=f"uq_{ps}_{bi}",
                        tag="uq",
                    )
                    nc.vector.tensor_tensor(
                        ua[:, :W], tp[:, 0, :W], tp[:, 1, :W], op=OP.add
                    )
                    nc.vector.tensor_tensor(
                        ub[:, :W], tp[:, 2, :W], tp[:, 3, :W], op=OP.add
                    )
                    nc.vector.tensor_tensor(
                        uq[:, :W], ua[:, :W], ub[:, :W], op=OP.add
                    )
                    unit = uq[:, :W]
                pend_srep.append((unit, bi == 0))

                if len(pend_srep) > 2:
                    flush_srep(False)
                if last_b:
                    while len(pend_srep) > 1:
                        flush_srep(False)
                    flush_srep(True)

                if ps == 0 and bi == 0:
                    for wi in range(24):
                        nc.tensor.matmul(
                            warm_ps[:], negones[:], negones[:],
                            start=(wi == 0), stop=(wi == 23),
                        )
                if not last_b:
                    continue
                # --- drain pass ps: PSUM -> SBUF -> DRAM -> RS -------
                srep_sb = msbp.tile(
                    [128, 512], F32, name=f"srep_sb_{ps}", tag="srep_sb",
                )
                nc.vector.tensor_copy(srep_sb[:, :W], srep[par][:, :W])
                for bti in range(2):
                    m_sb = msbp.tile(
                        [128, 512], F32, name=f"m_sb_{ps}_{bti}",
                        tag=f"m_sb{bti}",
                    )
                    nc.vector.tensor_tensor(
                        m_sb[:, :W], pk[(par, bti)][:, :W],
                        srep_sb[:, :W], op=OP.add,
                    )
                    nc.gpsimd.dma_start(
                        ar[ps][bti * 128:(bti + 1) * 128,
                               aroff[ps]:aroff[ps] + W],
                        m_sb[:, :W],
                    )
                    m_sb_last = m_sb
                if ps == 0:
                    continue  # merged with pass 1's RS
                nc.gpsimd.collective_compute(
                    "ReduceScatter",
                    OP.add,
                    replica_groups=[list(range(NCORES))],
                    ins=[ar[ps].opt()],
                    outs=[rs_t[ps].opt()],
                )

            # --- tail: msum loads + per-slice epilogues -------------
            # msum loads pinned behind the last pass's drain via a tiny
            # WAW copy so the scheduler can never hoist them; they ride
            # the sync queue, idle once att loads are done.
            msum = {}
            msum[1] = epp.tile([BS, PW[0] + PW[1]], F32, name="msum_a",
                               bufs=1)
            for p_i in range(2, P):
                msum[p_i] = epp.tile([BS, PW[p_i]], F32,
                                     name=f"msum_{p_i}", bufs=1)
            for p_i in sorted(msum):
                nc.vector.tensor_copy(
                    msum[p_i][0:BS, 0:1], m_sb_last[0:BS, 0:1]
                )
                nc.sync.dma_start(msum[p_i][:], rs_t[p_i][:])

            def emit_epi(col, mt, w, lab_off):
                nc.vector.tensor_reduce(
                    vcat[:, col:col + 1], mt[:], axis=AX, op=OP.max
                )
                cand = epp.tile(
                    [BS, w], F32, name=f"cand_{col}", tag=f"cand_{col}",
                )
                nc.vector.scalar_tensor_tensor(
                    cand[:], mt[:], vcat[:, col:col + 1],
                    lab1[:, lab_off:lab_off + w],
                    op0=OP.is_equal, op1=OP.mult,
                )
                nc.vector.tensor_reduce(
                    lcat[:, col:col + 1], cand[:], axis=AX, op=OP.max
                )

            emit_epi(0, msum[1], PW[0] + PW[1], 0)
            for p_i in range(2, P):
                emit_epi(p_i - 1, msum[p_i], PW[p_i], POFF[p_i])

            # --- final combine across slices ------------------------
            NS = P - 1
            vg = epp.tile([BS, 1], F32, bufs=1)
            nc.vector.tensor_reduce(vg[:], vcat[:, :NS], axis=AX, op=OP.max)
            candp = epp.tile([BS, NS], F32, bufs=1)
            nc.vector.scalar_tensor_tensor(
                candp[:], vcat[:, :NS], vg[:], lcat[:, :NS],
                op0=OP.is_equal, op1=OP.mult,
            )
            lmax = epp.tile([BS, 1], F32, bufs=1)
            nc.vector.tensor_reduce(lmax[:], candp[:], axis=AX, op=OP.max)
            labf = epp.tile([BS, 1], F32, bufs=1)
            nc.vector.tensor_scalar_add(labf[:], lmax[:], -1.0)
            labi = epp.tile([BS, 1], mybir.dt.int32, bufs=1)
            nc.vector.tensor_copy(labi[:], labf[:])
            nc.scalar.dma_start(out_d[:], labi[:])

    nc.compile()
    return nc


def shard_inputs(query, queue_anchor, queue_label, dsh=DSH, d_real=D):
    """Host-side layout prep: pad D with 1.0 (log 1 = 0); at in
    pass-major fp16 layout (one tensor per k-pass, [128, NT, W]),
    qt tile-major [128, NT, B]; label row replicated."""
    np_dt = np.float16
    q = np.asarray(query, np.float32)
    a = np.asarray(queue_anchor, np.float32)
    lab1 = (np.asarray(queue_label).astype(np.float32) + 1.0)[None, :]
    lab1 = np.ascontiguousarray(np.broadcast_to(lab1, (BS, lab1.shape[1])))
    in_maps = []
    for c in range(NCORES):
        lo = c * dsh
        hi = min((c + 1) * dsh, d_real)
        at = np.ones((dsh, a.shape[0]), np_dt)
        qt = np.ones((dsh, q.shape[0]), np_dt)
        if hi > lo:
            at[: hi - lo, :] = a[:, lo:hi].T.astype(np_dt)
            qt[: hi - lo, :] = q[:, lo:hi].T.astype(np_dt)
        # at: [dsh, K] -> per pass [128, NT, W] (tile-major, contiguous)
        at4 = at.reshape(NT, 128, K).transpose(1, 0, 2)  # [128, NT, K]
        m = {}
        for p in range(P):
            m[f"at{p}"] = np.ascontiguousarray(
                at4[:, :, POFF[p]:POFF[p] + PW[p]]
            )
        # qt: [dsh, B] -> tile-major [128, NT, B]
        m["qt"] = np.ascontiguousarray(
            qt.reshape(NT, 128, -1).transpose(1, 0, 2)
        )
        m["lab1"] = lab1
        in_maps.append(m)
    return in_maps


def unshard_out(per_core_outs, split_rs=False):
    """Reassemble the 8 cores' 32-label slices into the [256] output."""
    return np.concatenate([np.asarray(o) for o in per_core_outs])


_NC_CACHE = {}


def _split_rs_active():
    return False


def _get_nc():
    key = ("v3",)
    if key not in _NC_CACHE:
        _NC_CACHE[key] = build()
    return _NC_CACHE[key]


def kernel(query, queue_anchor, queue_label):
    nc = _get_nc()
    in_maps = shard_inputs(query, queue_anchor, queue_label)
    res = bass_utils.run_bass_kernel_spmd(
        nc, in_maps, core_ids=list(range(NCORES))
    )
    out = unshard_out([res.results[i]["out"] for i in range(NCORES)])
    return out.astype(np.asarray(queue_label).dtype)
